# revision 16
# baseline (speedup 1.0000x reference)
"""Multi-head causal attention (B=2, T=2048, D=1024, H=16, HD=64) on 8 TRN2
NeuronCores.

Sharding: head-pair x both batches. Core c handles heads {2c, 2c+1} for BOTH
batch elements. Wq/Wk/Wv are split column-wise (128 cols per core), Wo
row-wise; each core produces a full [T, D] partial output per batch (its 2
heads' contribution), which the host sums across all 8 cores per batch.

v3: two per-batch MEGAWINDOWS instead of four per-(batch,head) windows. The
two heads' score matmuls (K=64 half-array) are emitted as adjacent row-tiles
(tile_position (0,0) / (64,0), auto-derived from the KT/QT partition slices)
so the PE runs them CONCURRENTLY: ST wall time halves and the array sees
full-K activity, which keeps the HAM clock-gate at 8/8. Dense work (QKT
projections, V, out-projection tiles) is balanced across both megawindows so
neither the start nor the tail degenerates to half-array-only work:

  pre : QT(b0) c0-3 + KT(b0) c0                                (~20k cyc)
  W0  : STpair/CT(b0,h0+h1) + V(b0) + KT(b0)c1-3 + QT(b1)c0-2
        + V(b1)tt0-3 + out(b0)tt0-7                            (~105k cyc)
  W1  : STpair/CT(b1,h0+h1) + QT(b1)c3 + KT(b1) + V(b1)tt4-15
        + out(b0)tt8-15 + out(b1)tt0-10                        (~105k cyc)
  tail: out(b1) tt11-15 on the freed psST/psCT banks

Both megawindows use the front-loaded CT dribble (group j finishes at slot
4j+4) so out-projection tiles unblock mid-window. PSUM: 4 banks ST (2 tags x
2 bufs, [128,512]), 2 banks CT (per-head tags, 1 buf — the >=1-slot gap
between a group's stg copy and the next group's first matmul hides the WAR),
2 banks proj/out. A ~3.4us HAM warm-up block of matmuls on a zero tile runs
during the otherwise-dead input-DMA window. Exps all on ACT (~30us per
45us window); evacuation copies split DVE/ACT/gpsimd; norm-chain bounce DMAs
ride the gpsimd SWDGE queue; bulk x/out transfers stay on sync."""

import contextlib

import numpy as np

T, D = 2048, 1024
NH, HD = 16, 64
HPC = 2  # heads per core per batch
NB = 2  # batches (both on every core)
NCORES = 8
ND = D // 128  # 8 d-tiles
NT = T // 128  # 16 t/k-tiles
NQ = T // 512  # 4 q-chunks

_NC = None


def _build_nc():
    import concourse.mybir as mybir
    import concourse.tile as tile
    from concourse import bacc
    from concourse.masks import make_upper_triangular

    f32 = mybir.dt.float32
    bf16 = mybir.dt.bfloat16
    fp16 = mybir.dt.float16
    Exp = mybir.ActivationFunctionType.Exp

    nc = bacc.Bacc("TRN2", target_bir_lowering=False, debug=False, num_devices=NCORES)

    xT_d = [nc.dram_tensor(f"xT{b}", [D, T], bf16, kind="ExternalInput").ap() for b in range(NB)]
    wq_d = nc.dram_tensor("wq", [128, ND * 128], bf16, kind="ExternalInput").ap()
    wk_d = nc.dram_tensor("wk", [128, ND * 128], bf16, kind="ExternalInput").ap()
    wv_d = nc.dram_tensor("wv", [128, ND * 128], bf16, kind="ExternalInput").ap()
    wo_d = nc.dram_tensor("wo", [HPC * HD, D], bf16, kind="ExternalInput").ap()
    out_d = [nc.dram_tensor(f"out{b}", [T, D], fp16, kind="ExternalOutput").ap() for b in range(NB)]
    rscr = nc.dram_tensor("rscr", [128, 64], f32).ap()
    rscr2 = nc.dram_tensor("rscr2", [128, 64], f32).ap()

    with tile.TileContext(nc) as tc, contextlib.ExitStack() as ctx:
        pool = lambda **kw: ctx.enter_context(tc.tile_pool(**kw))
        constp = pool(name="const", bufs=1)
        qkp = pool(name="qk", bufs=1)
        vp = pool(name="vpool", bufs=1)
        wop = pool(name="wop", bufs=1)
        etp = pool(name="et", bufs=1)
        stgp = pool(name="stg", bufs=4)
        ctgp = pool(name="ctg", bufs=1)
        normp = pool(name="norm", bufs=2)
        rbp = pool(name="rb", bufs=6)
        ohp = pool(name="oh", bufs=4)
        bctx = contextlib.ExitStack()
        psST = bctx.enter_context(tc.tile_pool(name="psST", bufs=1, space="PSUM"))
        psCT = bctx.enter_context(tc.tile_pool(name="psCT", bufs=1, space="PSUM"))
        actx = contextlib.ExitStack()
        apool = lambda **kw: actx.enter_context(tc.tile_pool(**kw))
        xtp = apool(name="xtr", bufs=1)
        wtp = apool(name="wtiles", bufs=1)
        psProj = apool(name="psProj", bufs=2, space="PSUM")

        mask = constp.tile([128, 128], bf16, name="mask")
        make_upper_triangular(nc, mask[:], val=1.0, diag=True)

        # HAM warm-up: ~3.4us of back-to-back matmuls on a zero tile so the
        # PE clock is at K=8/8 by the time the first real matmul's DMA deps
        # land. Runs during the otherwise-dead input-DMA window.
        warm = constp.tile([128, 512], bf16, name="warm")
        nc.vector.memset(warm[:], 0.0)
        wps = [psProj.tile([128, 512], f32, name=f"warmps{i}", tag="proj") for i in range(2)]
        for i in range(8):
            nc.tensor.matmul(wps[i % 2][:], warm[:, 0:128], warm[:], start=True, stop=True)

        # QT/KT per batch: [128 = 2 heads x 64hd, T]
        QT = [qkp.tile([128, T], bf16, name=f"QT{b}") for b in range(NB)]
        KT = [qkp.tile([128, T], bf16, name=f"KT{b}") for b in range(NB)]
        # V natural per batch: [128 t, 66*HPC] with ones-columns
        vsb = [[vp.tile([128, 66 * HPC], bf16, name=f"v{b}_{tt}") for tt in range(NT)] for b in range(NB)]
        wo_sb = wop.tile([128, D], bf16, name="wo_sb")

        # ---------- loads ----------
        wsb = {}
        wtiles = {}

        def load_w(wname, wd):
            wsb[wname] = wtp.tile([128, ND * 128], bf16, name=f"{wname}sb", tag=f"{wname}sb")
            nc.sync.dma_start(wsb[wname][:], wd)
            wtiles[wname] = [wsb[wname][:, 128 * dt : 128 * (dt + 1)] for dt in range(ND)]

        xtr = [
            [xtp.tile([128, T], bf16, name=f"xtr{b}_{dt}", tag=f"xtr{b}_{dt}") for dt in range(ND)]
            for b in range(NB)
        ]
        # b0 x in half-tiles, dt-major per half, all on the sync queue.
        load_w("wq", wq_d)
        for dt in range(ND):
            nc.sync.dma_start(xtr[0][dt][:, 0:1024], xT_d[0][128 * dt : 128 * (dt + 1), 0:1024])
        load_w("wk", wk_d)
        for dt in range(ND):
            nc.sync.dma_start(xtr[0][dt][:, 1024:T], xT_d[0][128 * dt : 128 * (dt + 1), 1024:T])
        load_w("wv", wv_d)
        nc.sync.dma_start(wo_sb[:], wo_d)
        for dt in range(ND):
            nc.sync.dma_start(xtr[1][dt][:], xT_d[1][128 * dt : 128 * (dt + 1), :])

        # ---------- emission units ----------
        def emit_qkt_unit(wname, outs, b, c):
            ps = psProj.tile([128, 512], f32, name=f"pj_{wname}{b}_{c}", tag="proj")
            for dt in range(ND):
                nc.tensor.matmul(
                    ps[:],
                    wtiles[wname][dt][:],
                    xtr[b][dt][:, 512 * c : 512 * (c + 1)],
                    start=(dt == 0),
                    stop=(dt == ND - 1),
                )
            nc.vector.tensor_copy(outs[b][:, 512 * c : 512 * (c + 1)], ps[:])

        def emit_v(b, tt):
            ps = psProj.tile([128, 128], f32, name=f"vps{b}_{tt}", tag="proj")
            for dt in range(ND):
                nc.tensor.matmul(
                    ps[:],
                    xtr[b][dt][:, 128 * tt : 128 * (tt + 1)],
                    wtiles["wv"][dt][:],
                    start=(dt == 0),
                    stop=(dt == ND - 1),
                )
            nc.any.memset(vsb[b][tt][:, 64 : 66 * HPC : 66], 1.0)
            for h in range(HPC):
                nc.vector.tensor_copy(
                    vsb[b][tt][:, 66 * h : 66 * h + 64], ps[:, 64 * h : 64 * (h + 1)]
                )

        ets = {}  # (b, h, kt) -> ET tile

        def emit_st_alloc(b, kt):
            for h in range(HPC):
                ets[(b, h, kt)] = etp.tile(
                    [128, T - 128 * kt], bf16, name=f"et_b{b}h{h}_kt{kt}", tag=f"et{h}_{kt}"
                )

        def emit_st_part(b, kt, part):
            """Both heads' score matmuls for one 1024-col part of k-tile kt,
            emitted adjacently: h0 on PE rows 0-63 (tile_position (0,0)), h1
            on rows 64-127 ((64,0)) -- the HW runs each level of the pair
            concurrently. One [128,1024] PSUM tile per head (2 banks), one
            exp per head per part."""
            w = T - 128 * kt
            off = 1024 * part
            pw = min(1024, w - off)
            if pw <= 0:
                return
            pss = {}
            for h in range(HPC):
                pss[h] = psST.tile(
                    [128, pw], f32, name=f"st_b{b}h{h}_k{kt}_p{part}", tag=f"st{h}"
                )
            for c in range((pw + 511) // 512):
                n = min(512, pw - 512 * c)
                q0 = 128 * kt + off + 512 * c
                for h in range(HPC):
                    p0 = 64 * h
                    nc.tensor.matmul(
                        pss[h][:, 512 * c : 512 * c + n],
                        KT[b][p0 : p0 + 64, 128 * kt : 128 * (kt + 1)],
                        QT[b][p0 : p0 + 64, q0 : q0 + n],
                        start=True,
                        stop=True,
                    )
            for h in range(HPC):
                nc.scalar.activation(
                    ets[(b, h, kt)][:, off : off + pw],
                    pss[h][:, 0:pw],
                    Exp,
                    scale=0.125,
                )
            if part == 0:
                for h in range(HPC):
                    nc.gpsimd.tensor_mul(
                        ets[(b, h, kt)][:, 0:128], ets[(b, h, kt)][:, 0:128], mask[:]
                    )

        stg = {}
        ct_ps = {}

        def emit_ct_mms(b, h, j, kts, first, last):
            if first:
                ct_ps[(b, h, j)] = psCT.tile(
                    [65, 512], f32, name=f"ct_b{b}h{h}_j{j}", tag=f"ct{h}"
                )
            ct = ct_ps[(b, h, j)]
            for kt in kts:
                etoff = 512 * j - 128 * kt
                if etoff >= 0:
                    n, psoff, ecol = 512, 0, etoff
                else:
                    n, psoff, ecol = 512 + etoff, -etoff, 0
                nc.tensor.matmul(
                    ct[0:65, psoff : psoff + n],
                    vsb[b][kt][:, 66 * h : 66 * h + 65],
                    ets[(b, h, kt)][:, ecol : ecol + n],
                    start=(kt == 0),
                    stop=(last and kt == kts[-1]),
                )

        def finish_ct(b, h, j):
            ct = ct_ps[(b, h, j)]
            s = stgp.tile([65, 512], f32, name=f"stg_b{b}h{h}_j{j}", tag="stg")
            stg[(b, h, j)] = s
            nc.vector.tensor_copy(s[:], ct[:])

        CTG = [ctgp.tile([128, T], bf16, name=f"ctg{b}") for b in range(NB)]

        def emit_norm(b, h, j):
            idx = 8 * b + 4 * h + j
            # reciprocal of the ones-column row-sums (partition 64 of stg),
            # broadcast to 64 partitions via the gpsimd custom instruction --
            # no DRAM bounce, no tiny DMAs on a shared queue.
            rc_hj = normp.tile([1, 512], f32, name=f"rc{idx}", tag="rc")
            nc.vector.reciprocal(rc_hj[:], stg[(b, h, j)][64:65, :])
            rb = rbp.tile([64, 512], f32, name=f"rb{idx}", tag="rb")
            nc.gpsimd.partition_broadcast(rb[:], rc_hj[:])
            eng = nc.vector if j in (0, 3) else nc.gpsimd
            eng.tensor_mul(
                CTG[b][64 * h : 64 * h + 64, 512 * j : 512 * (j + 1)],
                stg[(b, h, j)][0:64, :],
                rb[:],
            )

        # ---------- CT dribble: strictly sequential groups ----------
        # group j's k-tiles {0..4j+3} spread over slots 4j+1..4j+4 in
        # (j+1)-sized chunks; group 3 over slots 13-15 (6/5/5). Exactly one
        # open accumulation group per head at any time, so psCT needs just
        # one bank per head, and the >=1-slot gap between a group's stg
        # copy and the next group's first matmul hides the WAR.
        drib = {sw: [] for sw in range(NT)}
        for j in range(3):
            kts = list(range(4 * j + 4))
            for sl in range(4):
                chunk = kts[(j + 1) * sl : (j + 1) * (sl + 1)]
                drib[sl + 1 + j * 4].append((j, chunk, sl == 0, sl == 3))
        drib[13].append((3, list(range(0, 6)), True, False))
        drib[14].append((3, list(range(6, 11)), False, False))
        drib[15].append((3, list(range(11, 16)), False, True))

        # ---------- out-projection unit ----------
        psO_holder = {"pool": psProj, "tag": ["proj", "proj"]}

        def emit_out(b, tt, copy_eng=None, dma_eng=None):
            oh = ohp.tile([128, D], fp16, name=f"oh{b}_{tt}", tag="oh")
            for dc in range(2):
                ps = psO_holder["pool"].tile(
                    [128, 512], f32, name=f"ops{b}_{tt}_{dc}", tag=psO_holder["tag"][dc]
                )
                nc.tensor.matmul(
                    ps[:],
                    CTG[b][:, 128 * tt : 128 * (tt + 1)],
                    wo_sb[:, 512 * dc : 512 * (dc + 1)],
                    start=True,
                    stop=True,
                )
                eng = copy_eng or nc.vector
                if eng is nc.scalar:
                    eng.copy(oh[:, 512 * dc : 512 * (dc + 1)], ps[:])
                else:
                    eng.tensor_copy(oh[:, 512 * dc : 512 * (dc + 1)], ps[:])
            (dma_eng or nc.sync).dma_start(out_d[b][128 * tt : 128 * (tt + 1), :], oh[:])

        # ---------- dense-unit schedule ----------
        # Each slot: (pre, post) dense-unit lists; pre runs before the ST
        # pair (needed when the ST pair itself depends on the unit).
        # W0: V(b0) every slot; KT(b0)c1-3 slots 1-3; QT(b1)c0-2 slots
        #     9/11/13 (after the b1 input stream lands); V(b1)tt0-3 slots
        #     12-15; out(b0)tt0-7 slots 7-14.
        w0_pre = {sw: [] for sw in range(NT)}
        w0_post = {sw: [] for sw in range(NT)}
        for sw in range(NT):
            w0_post[sw].append(("v", 0, sw))
        for c in range(1, 4):
            w0_post[c].append(("qkt", "wk", KT, 0, c))
        for c in range(3):
            w0_post[9 + 2 * c].append(("qkt", "wq", QT, 1, c))
        for i in range(4):
            w0_post[12 + i].append(("v", 1, i))
        for i in range(8):
            w0_post[7 + i].append(("out", 0, i))

        # W1: QT(b1)c3 + KT(b1)c0 before slot 0's ST pair; KT(b1)c1-3 slots
        #     1-3; V(b1)tt4-15 slots 0-11; out(b0)tt8-15 slots 0-7;
        #     out(b1)tt0-10 slots 8-15.
        w1_pre = {sw: [] for sw in range(NT)}
        w1_post = {sw: [] for sw in range(NT)}
        w1_pre[0].append(("qkt", "wq", QT, 1, 3))
        w1_pre[0].append(("qkt", "wk", KT, 1, 0))
        for c in range(1, 4):
            w1_post[c].append(("qkt", "wk", KT, 1, c))
        for i in range(12):
            w1_post[i].append(("v", 1, 4 + i))
        for i in range(8):
            w1_post[i].append(("out", 0, 8 + i))
        for i in range(4):
            w1_post[8 + i].append(("out", 1, i))
        for i in range(4):
            w1_post[11 + i].append(("out", 1, 4 + i))
        w1_post[14].append(("out", 1, 8))
        w1_post[15].append(("out", 1, 9))
        w1_post[15].append(("out", 1, 10))
        dense_pre = [w0_pre, w1_pre]
        dense_post = [w0_post, w1_post]

        def run_dense(ent, sw):
            kind = ent[0]
            if kind == "qkt":
                emit_qkt_unit(ent[1], ent[2], ent[3], ent[4])
            elif kind == "v":
                emit_v(ent[1], ent[2])
            else:
                b, tt = ent[1], ent[2]
                copy_eng = [nc.vector, nc.scalar][(tt + b) % 2]
                emit_out(b, tt, copy_eng=copy_eng)

        # ---------- schedule ----------
        emit_qkt_unit("wq", QT, 0, 0)
        emit_qkt_unit("wq", QT, 0, 1)
        emit_qkt_unit("wk", KT, 0, 0)
        emit_qkt_unit("wq", QT, 0, 2)
        emit_qkt_unit("wq", QT, 0, 3)

        for b in range(NB):
            for sw in range(NT):
                for ent in dense_pre[b][sw]:
                    run_dense(ent, sw)
                emit_st_alloc(b, sw)
                emit_st_part(b, sw, 0)
                for j, kts_, first, last in drib[sw]:
                    for h in range(HPC):
                        emit_ct_mms(b, h, j, kts_, first, last)
                        if last:
                            finish_ct(b, h, j)
                            emit_norm(b, h, j)
                emit_st_part(b, sw, 1)
                for ent in dense_post[b][sw]:
                    run_dense(ent, sw)

        # tail: remaining b1 out tiles on the freed psST banks
        psO_holder["pool"] = psST
        psO_holder["tag"] = ["st0", "st1"]
        tail_copy = [nc.scalar, nc.vector, nc.scalar, nc.vector, nc.scalar]
        for i, tt in enumerate(range(11, 16)):
            emit_out(1, tt, copy_eng=tail_copy[i], dma_eng=[nc.sync, nc.gpsimd][i % 2])
        actx.close()
        bctx.close()

    nc.compile()
    return nc


def _get_nc():
    global _NC
    if _NC is None:
        _NC = _build_nc()
    return _NC


def make_in_maps(x, wq, wk, wv, wo):
    import ml_dtypes

    bf = ml_dtypes.bfloat16

    def rearr(w, cs):
        # [D, 128] -> [128, ND*128] with (p, dt, c) layout for linear DMA
        return np.ascontiguousarray(
            w[:, cs].reshape(ND, 128, 128).transpose(1, 0, 2).reshape(128, ND * 128)
        ).astype(bf)

    xT = [np.ascontiguousarray(x[b].T).astype(bf) for b in range(NB)]
    in_maps = []
    for c in range(NCORES):
        cs = slice(128 * c, 128 * (c + 1))
        in_maps.append(
            {
                "xT0": xT[0],
                "xT1": xT[1],
                "wq": rearr(wq, cs),
                "wk": rearr(wk, cs),
                "wv": rearr(wv, cs),
                "wo": np.ascontiguousarray(wo[cs, :]).astype(bf),
            }
        )
    return in_maps


def kernel(x, wq, wk, wv, wo, bo):
    from concourse.bass_utils import run_bass_kernel_spmd

    x = np.asarray(x, dtype=np.float32)
    wq = np.asarray(wq, dtype=np.float32)
    wk = np.asarray(wk, dtype=np.float32)
    wv = np.asarray(wv, dtype=np.float32)
    wo = np.asarray(wo, dtype=np.float32)
    bo = np.asarray(bo, dtype=np.float32)

    nc = _get_nc()
    in_maps = make_in_maps(x, wq, wk, wv, wo)
    try:
        res = run_bass_kernel_spmd(nc, in_maps, core_ids=list(range(NCORES))).results
    except Exception:
        res = run_bass_kernel_spmd(nc, in_maps, core_ids=list(range(NCORES))).results
    out = np.zeros((2, T, D), dtype=np.float32)
    for c in range(NCORES):
        for b in range(NB):
            out[b] += res[c][f"out{b}"].astype(np.float32)
    out += bo[None, None, :]
    return out


# revision 18
# speedup vs baseline: 1.7241x; 1.7241x over previous
"""Multi-head causal attention (B=2, T=2048, D=1024, H=16, HD=64) on 8 TRN2
NeuronCores.

Sharding: head-pair x both batches. Core c handles heads {2c, 2c+1} for BOTH
batch elements. Wq/Wk/Wv are split column-wise (128 cols per core), Wo
row-wise; each core produces a full [T, D] partial output per batch (its 2
heads' contribution), which the host sums across all 8 cores per batch.

v3: two per-batch MEGAWINDOWS instead of four per-(batch,head) windows. The
two heads' score matmuls (K=64 half-array) are emitted as adjacent row-tiles
(tile_position (0,0) / (64,0), auto-derived from the KT/QT partition slices)
so the PE runs them CONCURRENTLY: ST wall time halves and the array sees
full-K activity, which keeps the HAM clock-gate at 8/8. Dense work (QKT
projections, V, out-projection tiles) is balanced across both megawindows so
neither the start nor the tail degenerates to half-array-only work:

  pre : QT(b0) c0-3 + KT(b0) c0                                (~20k cyc)
  W0  : STpair/CT(b0,h0+h1) + V(b0) + KT(b0)c1-3 + QT(b1)c0-2
        + V(b1)tt0-3 + out(b0)tt0-7                            (~105k cyc)
  W1  : STpair/CT(b1,h0+h1) + QT(b1)c3 + KT(b1) + V(b1)tt4-15
        + out(b0)tt8-15 + out(b1)tt0-10                        (~105k cyc)
  tail: out(b1) tt11-15 on the freed psST/psCT banks

Both megawindows use the front-loaded CT dribble (group j finishes at slot
4j+4) so out-projection tiles unblock mid-window. PSUM: 4 banks ST (2 tags x
2 bufs, [128,512]), 2 banks CT (per-head tags, 1 buf — the >=1-slot gap
between a group's stg copy and the next group's first matmul hides the WAR),
2 banks proj/out. A ~3.4us HAM warm-up block of matmuls on a zero tile runs
during the otherwise-dead input-DMA window. Exps all on ACT (~30us per
45us window); evacuation copies split DVE/ACT/gpsimd; norm-chain bounce DMAs
ride the gpsimd SWDGE queue; bulk x/out transfers stay on sync."""

import contextlib

import numpy as np

T, D = 2048, 1024
NH, HD = 16, 64
HPC = 2  # heads per core per batch
NB = 2  # batches (both on every core)
NCORES = 8
ND = D // 128  # 8 d-tiles
NT = T // 128  # 16 t/k-tiles
NQ = T // 512  # 4 q-chunks

_NC = None


def _build_nc():
    import concourse.mybir as mybir
    import concourse.tile as tile
    from concourse import bacc
    from concourse.masks import make_upper_triangular

    f32 = mybir.dt.float32
    bf16 = mybir.dt.bfloat16
    fp16 = mybir.dt.float16
    Exp = mybir.ActivationFunctionType.Exp

    nc = bacc.Bacc("TRN2", target_bir_lowering=False, debug=False, num_devices=NCORES)

    xT_d = [nc.dram_tensor(f"xT{b}", [D, T], bf16, kind="ExternalInput").ap() for b in range(NB)]
    wq_d = nc.dram_tensor("wq", [128, ND * 128], bf16, kind="ExternalInput").ap()
    wk_d = nc.dram_tensor("wk", [128, ND * 128], bf16, kind="ExternalInput").ap()
    wv_d = nc.dram_tensor("wv", [128, ND * 128], bf16, kind="ExternalInput").ap()
    wo_d = nc.dram_tensor("wo", [HPC * HD, D], bf16, kind="ExternalInput").ap()
    out_d = [nc.dram_tensor(f"out{b}", [T, D], fp16, kind="ExternalOutput").ap() for b in range(NB)]
    rscr = nc.dram_tensor("rscr", [128, 64], f32).ap()
    rscr2 = nc.dram_tensor("rscr2", [128, 64], f32).ap()

    with tile.TileContext(nc) as tc, contextlib.ExitStack() as ctx:
        pool = lambda **kw: ctx.enter_context(tc.tile_pool(**kw))
        constp = pool(name="const", bufs=1)
        qkp = pool(name="qk", bufs=1)
        vp = pool(name="vpool", bufs=1)
        wop = pool(name="wop", bufs=1)
        etp = pool(name="et", bufs=1)
        stgp = pool(name="stg", bufs=4)
        ctgp = pool(name="ctg", bufs=1)
        normp = pool(name="norm", bufs=2)
        rbp = pool(name="rb", bufs=6)
        ohp = pool(name="oh", bufs=4)
        bctx = contextlib.ExitStack()
        psST = bctx.enter_context(tc.tile_pool(name="psST", bufs=1, space="PSUM"))
        psCT = bctx.enter_context(tc.tile_pool(name="psCT", bufs=1, space="PSUM"))
        actx = contextlib.ExitStack()
        apool = lambda **kw: actx.enter_context(tc.tile_pool(**kw))
        xtp = apool(name="xtr", bufs=1)
        wtp = apool(name="wtiles", bufs=1)
        psProj = apool(name="psProj", bufs=2, space="PSUM")

        mask = constp.tile([128, 128], bf16, name="mask")
        make_upper_triangular(nc, mask[:], val=1.0, diag=True)

        # HAM warm-up: ~3.4us of back-to-back matmuls on a zero tile so the
        # PE clock is at K=8/8 by the time the first real matmul's DMA deps
        # land. Runs during the otherwise-dead input-DMA window.
        warm = constp.tile([128, 512], bf16, name="warm")
        nc.vector.memset(warm[:], 0.0)
        wps = [psProj.tile([128, 512], f32, name=f"warmps{i}", tag="proj") for i in range(2)]
        for i in range(8):
            nc.tensor.matmul(wps[i % 2][:], warm[:, 0:128], warm[:], start=True, stop=True)

        # QT/KT per batch: [128 = 2 heads x 64hd, T]
        QT = [qkp.tile([128, T], bf16, name=f"QT{b}") for b in range(NB)]
        KT = [qkp.tile([128, T], bf16, name=f"KT{b}") for b in range(NB)]
        # V natural per batch: [128 t, 66*HPC] with ones-columns
        vsb = [[vp.tile([128, 66 * HPC], bf16, name=f"v{b}_{tt}") for tt in range(NT)] for b in range(NB)]
        wo_sb = wop.tile([128, D], bf16, name="wo_sb")

        # ---------- loads ----------
        wsb = {}
        wtiles = {}

        def load_w(wname, wd):
            wsb[wname] = wtp.tile([128, ND * 128], bf16, name=f"{wname}sb", tag=f"{wname}sb")
            nc.sync.dma_start(wsb[wname][:], wd)
            wtiles[wname] = [wsb[wname][:, 128 * dt : 128 * (dt + 1)] for dt in range(ND)]

        xtr = [
            [xtp.tile([128, T], bf16, name=f"xtr{b}_{dt}", tag=f"xtr{b}_{dt}") for dt in range(ND)]
            for b in range(NB)
        ]
        # b0 x in half-tiles, dt-major per half, all on the sync queue.
        load_w("wq", wq_d)
        for dt in range(ND):
            nc.sync.dma_start(xtr[0][dt][:, 0:1024], xT_d[0][128 * dt : 128 * (dt + 1), 0:1024])
        load_w("wk", wk_d)
        for dt in range(ND):
            nc.sync.dma_start(xtr[0][dt][:, 1024:T], xT_d[0][128 * dt : 128 * (dt + 1), 1024:T])
        load_w("wv", wv_d)
        nc.sync.dma_start(wo_sb[:], wo_d)
        for dt in range(ND):
            nc.sync.dma_start(xtr[1][dt][:], xT_d[1][128 * dt : 128 * (dt + 1), :])

        # ---------- emission units ----------
        def emit_qkt_unit(wname, outs, b, c):
            ps = psProj.tile([128, 512], f32, name=f"pj_{wname}{b}_{c}", tag="proj")
            for dt in range(ND):
                nc.tensor.matmul(
                    ps[:],
                    wtiles[wname][dt][:],
                    xtr[b][dt][:, 512 * c : 512 * (c + 1)],
                    start=(dt == 0),
                    stop=(dt == ND - 1),
                )
            nc.vector.tensor_copy(outs[b][:, 512 * c : 512 * (c + 1)], ps[:])

        def emit_v(b, tt):
            ps = psProj.tile([128, 128], f32, name=f"vps{b}_{tt}", tag="proj")
            for dt in range(ND):
                nc.tensor.matmul(
                    ps[:],
                    xtr[b][dt][:, 128 * tt : 128 * (tt + 1)],
                    wtiles["wv"][dt][:],
                    start=(dt == 0),
                    stop=(dt == ND - 1),
                )
            nc.any.memset(vsb[b][tt][:, 64 : 66 * HPC : 66], 1.0)
            for h in range(HPC):
                nc.vector.tensor_copy(
                    vsb[b][tt][:, 66 * h : 66 * h + 64], ps[:, 64 * h : 64 * (h + 1)]
                )

        ets = {}  # (b, h, kt) -> ET tile

        def emit_st_alloc(b, kt):
            for h in range(HPC):
                ets[(b, h, kt)] = etp.tile(
                    [128, T - 128 * kt], bf16, name=f"et_b{b}h{h}_kt{kt}", tag=f"et{h}_{kt}"
                )

        def emit_st_part(b, kt, part):
            """Both heads' score matmuls for one 1024-col part of k-tile kt,
            emitted adjacently: h0 on PE rows 0-63 (tile_position (0,0)), h1
            on rows 64-127 ((64,0)) -- the HW runs each level of the pair
            concurrently. One [128,1024] PSUM tile per head (2 banks), one
            exp per head per part."""
            w = T - 128 * kt
            off = 1024 * part
            pw = min(1024, w - off)
            if pw <= 0:
                return
            pss = {}
            for h in range(HPC):
                pss[h] = psST.tile(
                    [128, pw], f32, name=f"st_b{b}h{h}_k{kt}_p{part}", tag=f"st{h}"
                )
            for c in range((pw + 511) // 512):
                n = min(512, pw - 512 * c)
                q0 = 128 * kt + off + 512 * c
                for h in range(HPC):
                    p0 = 64 * h
                    nc.tensor.matmul(
                        pss[h][:, 512 * c : 512 * c + n],
                        KT[b][p0 : p0 + 64, 128 * kt : 128 * (kt + 1)],
                        QT[b][p0 : p0 + 64, q0 : q0 + n],
                        start=True,
                        stop=True,
                    )
            for h in range(HPC):
                nc.scalar.activation(
                    ets[(b, h, kt)][:, off : off + pw],
                    pss[h][:, 0:pw],
                    Exp,
                    scale=0.125,
                )
            if part == 0:
                for h in range(HPC):
                    nc.gpsimd.tensor_mul(
                        ets[(b, h, kt)][:, 0:128], ets[(b, h, kt)][:, 0:128], mask[:]
                    )

        stg = {}
        ct_ps = {}

        def emit_ct_mms(b, h, j, kts, first, last):
            if first:
                ct_ps[(b, h, j)] = psCT.tile(
                    [65, 512], f32, name=f"ct_b{b}h{h}_j{j}", tag=f"ct{h}"
                )
            ct = ct_ps[(b, h, j)]
            for kt in kts:
                etoff = 512 * j - 128 * kt
                if etoff >= 0:
                    n, psoff, ecol = 512, 0, etoff
                else:
                    n, psoff, ecol = 512 + etoff, -etoff, 0
                nc.tensor.matmul(
                    ct[0:65, psoff : psoff + n],
                    vsb[b][kt][:, 66 * h : 66 * h + 65],
                    ets[(b, h, kt)][:, ecol : ecol + n],
                    start=(kt == 0),
                    stop=(last and kt == kts[-1]),
                )

        def finish_ct(b, h, j):
            ct = ct_ps[(b, h, j)]
            s = stgp.tile([65, 512], f32, name=f"stg_b{b}h{h}_j{j}", tag="stg")
            stg[(b, h, j)] = s
            nc.vector.tensor_copy(s[:], ct[:])
            idx = 8 * b + 4 * h + j
            nc.gpsimd.dma_start(rscr[8 * idx : 8 * idx + 8, :], s[64:65, :])

        CTG = [ctgp.tile([128, T], bf16, name=f"ctg{b}") for b in range(NB)]
        rscr2v = rscr2.rearrange("(r p) c -> r (p c)", p=8)  # [16, 512] view

        def emit_norm(b, h, j):
            idx = 8 * b + 4 * h + j
            rs_hj = normp.tile([8, 64], f32, name=f"rs{idx}", tag="rs")
            nc.gpsimd.dma_start(rs_hj[:], rscr[8 * idx : 8 * idx + 8, :])
            rc_hj = normp.tile([8, 64], f32, name=f"rc{idx}", tag="rc")
            nc.vector.reciprocal(rc_hj[:], rs_hj[:])
            nc.gpsimd.dma_start(rscr2[8 * idx : 8 * idx + 8, :], rc_hj[:])
            rb = rbp.tile([64, 512], f32, name=f"rb{idx}", tag="rb")
            nc.gpsimd.dma_start(rb[:], rscr2v[idx : idx + 1, :].partition_broadcast(64))
            eng = nc.vector if j in (0, 3) else nc.gpsimd
            eng.tensor_mul(
                CTG[b][64 * h : 64 * h + 64, 512 * j : 512 * (j + 1)],
                stg[(b, h, j)][0:64, :],
                rb[:],
            )

        # ---------- CT dribble: strictly sequential groups ----------
        # group j's k-tiles {0..4j+3} spread over slots 4j+1..4j+4 in
        # (j+1)-sized chunks; group 3 over slots 13-15 (6/5/5). Exactly one
        # open accumulation group per head at any time, so psCT needs just
        # one bank per head, and the >=1-slot gap between a group's stg
        # copy and the next group's first matmul hides the WAR.
        drib = {sw: [] for sw in range(NT)}
        for j in range(3):
            kts = list(range(4 * j + 4))
            for sl in range(4):
                chunk = kts[(j + 1) * sl : (j + 1) * (sl + 1)]
                drib[sl + 1 + j * 4].append((j, chunk, sl == 0, sl == 3))
        drib[13].append((3, list(range(0, 6)), True, False))
        drib[14].append((3, list(range(6, 11)), False, False))
        drib[15].append((3, list(range(11, 16)), False, True))

        # ---------- out-projection unit ----------
        psO_holder = {"pool": psProj, "tag": ["proj", "proj"]}

        def emit_out(b, tt, copy_eng=None, dma_eng=None):
            oh = ohp.tile([128, D], fp16, name=f"oh{b}_{tt}", tag="oh")
            for dc in range(2):
                ps = psO_holder["pool"].tile(
                    [128, 512], f32, name=f"ops{b}_{tt}_{dc}", tag=psO_holder["tag"][dc]
                )
                nc.tensor.matmul(
                    ps[:],
                    CTG[b][:, 128 * tt : 128 * (tt + 1)],
                    wo_sb[:, 512 * dc : 512 * (dc + 1)],
                    start=True,
                    stop=True,
                )
                eng = copy_eng or nc.vector
                if eng is nc.scalar:
                    eng.copy(oh[:, 512 * dc : 512 * (dc + 1)], ps[:])
                else:
                    eng.tensor_copy(oh[:, 512 * dc : 512 * (dc + 1)], ps[:])
            (dma_eng or nc.sync).dma_start(out_d[b][128 * tt : 128 * (tt + 1), :], oh[:])

        # ---------- dense-unit schedule ----------
        # Each slot: (pre, post) dense-unit lists; pre runs before the ST
        # pair (needed when the ST pair itself depends on the unit).
        # W0: V(b0) every slot; KT(b0)c1-3 slots 1-3; QT(b1)c0-2 slots
        #     9/11/13 (after the b1 input stream lands); V(b1)tt0-3 slots
        #     12-15; out(b0)tt0-7 slots 7-14.
        w0_pre = {sw: [] for sw in range(NT)}
        w0_post = {sw: [] for sw in range(NT)}
        for sw in range(NT):
            w0_post[sw].append(("v", 0, sw))
        for c in range(1, 4):
            w0_post[c].append(("qkt", "wk", KT, 0, c))
        for c in range(3):
            w0_post[9 + 2 * c].append(("qkt", "wq", QT, 1, c))
        for i in range(4):
            w0_post[12 + i].append(("v", 1, i))
        for i in range(8):
            w0_post[7 + i].append(("out", 0, i))

        # W1: QT(b1)c3 + KT(b1)c0 before slot 0's ST pair; KT(b1)c1-3 slots
        #     1-3; V(b1)tt4-15 slots 0-11; out(b0)tt8-15 slots 0-7;
        #     out(b1)tt0-10 slots 8-15.
        w1_pre = {sw: [] for sw in range(NT)}
        w1_post = {sw: [] for sw in range(NT)}
        w1_pre[0].append(("qkt", "wq", QT, 1, 3))
        w1_pre[0].append(("qkt", "wk", KT, 1, 0))
        for c in range(1, 4):
            w1_post[c].append(("qkt", "wk", KT, 1, c))
        for i in range(12):
            w1_post[i].append(("v", 1, 4 + i))
        for i in range(8):
            w1_post[i].append(("out", 0, 8 + i))
        for i in range(4):
            w1_post[8 + i].append(("out", 1, i))
        for i in range(4):
            w1_post[11 + i].append(("out", 1, 4 + i))
        w1_post[14].append(("out", 1, 8))
        w1_post[15].append(("out", 1, 9))
        w1_post[15].append(("out", 1, 10))
        dense_pre = [w0_pre, w1_pre]
        dense_post = [w0_post, w1_post]

        def run_dense(ent, sw):
            kind = ent[0]
            if kind == "qkt":
                emit_qkt_unit(ent[1], ent[2], ent[3], ent[4])
            elif kind == "v":
                emit_v(ent[1], ent[2])
            else:
                b, tt = ent[1], ent[2]
                copy_eng = [nc.vector, nc.scalar][(tt + b) % 2]
                emit_out(b, tt, copy_eng=copy_eng)

        # ---------- schedule ----------
        emit_qkt_unit("wq", QT, 0, 0)
        emit_qkt_unit("wq", QT, 0, 1)
        emit_qkt_unit("wk", KT, 0, 0)
        emit_qkt_unit("wq", QT, 0, 2)
        emit_qkt_unit("wq", QT, 0, 3)

        for b in range(NB):
            for sw in range(NT):
                for ent in dense_pre[b][sw]:
                    run_dense(ent, sw)
                emit_st_alloc(b, sw)
                emit_st_part(b, sw, 0)
                # V units must precede the dribble: slot 15's group-3 chunk
                # consumes vsb[b][15] in the same slot.
                for ent in dense_post[b][sw]:
                    if ent[0] == "v":
                        run_dense(ent, sw)
                for j, kts_, first, last in drib[sw]:
                    for h in range(HPC):
                        emit_ct_mms(b, h, j, kts_, first, last)
                        if last:
                            finish_ct(b, h, j)
                            emit_norm(b, h, j)
                emit_st_part(b, sw, 1)
                for ent in dense_post[b][sw]:
                    if ent[0] != "v":
                        run_dense(ent, sw)

        # tail: remaining b1 out tiles on the freed psST banks
        psO_holder["pool"] = psST
        psO_holder["tag"] = ["st0", "st1"]
        tail_copy = [nc.scalar, nc.vector, nc.scalar, nc.vector, nc.scalar]
        for i, tt in enumerate(range(11, 16)):
            emit_out(1, tt, copy_eng=tail_copy[i], dma_eng=[nc.sync, nc.gpsimd][i % 2])
        actx.close()
        bctx.close()

    nc.compile()
    return nc


def _get_nc():
    global _NC
    if _NC is None:
        _NC = _build_nc()
    return _NC


def make_in_maps(x, wq, wk, wv, wo):
    import ml_dtypes

    bf = ml_dtypes.bfloat16

    def rearr(w, cs):
        # [D, 128] -> [128, ND*128] with (p, dt, c) layout for linear DMA
        return np.ascontiguousarray(
            w[:, cs].reshape(ND, 128, 128).transpose(1, 0, 2).reshape(128, ND * 128)
        ).astype(bf)

    xT = [np.ascontiguousarray(x[b].T).astype(bf) for b in range(NB)]
    in_maps = []
    for c in range(NCORES):
        cs = slice(128 * c, 128 * (c + 1))
        in_maps.append(
            {
                "xT0": xT[0],
                "xT1": xT[1],
                "wq": rearr(wq, cs),
                "wk": rearr(wk, cs),
                "wv": rearr(wv, cs),
                "wo": np.ascontiguousarray(wo[cs, :]).astype(bf),
            }
        )
    return in_maps


def kernel(x, wq, wk, wv, wo, bo):
    from concourse.bass_utils import run_bass_kernel_spmd

    x = np.asarray(x, dtype=np.float32)
    wq = np.asarray(wq, dtype=np.float32)
    wk = np.asarray(wk, dtype=np.float32)
    wv = np.asarray(wv, dtype=np.float32)
    wo = np.asarray(wo, dtype=np.float32)
    bo = np.asarray(bo, dtype=np.float32)

    nc = _get_nc()
    in_maps = make_in_maps(x, wq, wk, wv, wo)
    try:
        res = run_bass_kernel_spmd(nc, in_maps, core_ids=list(range(NCORES))).results
    except Exception:
        res = run_bass_kernel_spmd(nc, in_maps, core_ids=list(range(NCORES))).results
    out = np.zeros((2, T, D), dtype=np.float32)
    for c in range(NCORES):
        for b in range(NB):
            out[b] += res[c][f"out{b}"].astype(np.float32)
    out += bo[None, None, :]
    return out
